# revision 38
# baseline (speedup 1.0000x reference)
"""Trainium2 Bass kernel for the BN-attention module (nn_Attention).

Full inputs -> full output. Sharding: 8 cores = (batch b in 0..3) x
(head-group g in 0..1, 4 heads each). Each core computes its batch's
4-head attention and a partial output projection; the host sums the two
head-group partials per batch and adds the projection BN bias.

Numerics: BN scales are folded into the weights on the host. QK^T and
the projections run as float32r (fp32 storage, FP22 multiply, fp32
accumulate; full PE rate at free dim >= 256). Softmax skips the
max-subtraction (logits are O(25) here, safe in fp32); exp(S^T) is
stored bf16, and both attn@V and the softmax denominators consume the
same bf16 values so their rounding largely cancels in the ratio.

Layout: attention is computed transposed, S^T = K^T Q with keys (m) on
partitions, so attn@V needs no transposes at all: V is produced
directly as vT[n,d] by the projection, and the denominators are
column sums done with ones-matmuls on the TensorE (plus one level of
bf16 pair-summing on the VectorE, which runs at 2x for bf16).
Normalization, BN-v bias, and ReLU fold into the PSUM eviction of xx.
"""

import numpy as np

import concourse.bacc as bacc
import concourse.mybir as mybir
import concourse.tile as tile
from concourse.bass_utils import run_bass_kernel_spmd

# Problem dims (hardcoded per the spec)
B, C, H, W = 4, 256, 48, 48
N = H * W            # 2304
KD, NH, AR = 32, 8, 4
D = AR * KD          # 128 value dims per head
NHKD = NH * KD       # 256
DH = NH * D          # 1024
EPS = 1e-5

NHG = 4              # heads per core
J = 256              # n-chunk width
NJ = N // J          # 9
MT = 128             # m-tile (key tile)
NMT = N // MT        # 18

F32 = mybir.dt.float32
F32R = mybir.dt.float32r
BF16 = mybir.dt.bfloat16
AF = mybir.ActivationFunctionType
OP = mybir.AluOpType

# chunks of 2304 by <=512 for the projection matmuls
CHUNKS_512 = [(off, min(512, N - off)) for off in range(0, N, 512)]

_CACHE = {}


def _build_program():
    nc = bacc.Bacc("TRN2", target_bir_lowering=False, debug=False)

    x_in = nc.dram_tensor("x_in", [C, N], F32R, kind="ExternalInput")
    wqt_d = nc.dram_tensor("wqt", [C, 128], F32R, kind="ExternalInput")
    wkt_d = nc.dram_tensor("wkt", [C, 128], F32R, kind="ExternalInput")
    wvt_d = nc.dram_tensor("wvt", [C, 512], F32R, kind="ExternalInput")
    wpt_d = nc.dram_tensor("wpt", [512, C], F32R, kind="ExternalInput")
    bq_d = nc.dram_tensor("bq", [128, 1], F32, kind="ExternalInput")
    bk_d = nc.dram_tensor("bk", [128, 1], F32, kind="ExternalInput")
    bv_d = nc.dram_tensor("bv", [512, 1], F32, kind="ExternalInput")
    consts_d = nc.dram_tensor("consts", [128, 128 + J], F32R, kind="ExternalInput")
    ones_bf_d = nc.dram_tensor("ones_bf", [128, 1], BF16, kind="ExternalInput")
    out_d = nc.dram_tensor("outp", [C, N], F32, kind="ExternalOutput")

    with tile.TileContext(nc) as tc:
        with nc.allow_low_precision(reason="float32r rounding is intentional"), \
             tc.tile_pool(name="const", bufs=1) as constp, \
             tc.tile_pool(name="qk", bufs=1) as qkp, \
             tc.tile_pool(name="vt", bufs=1) as vtp, \
             tc.tile_pool(name="pexp", bufs=1) as pexpp, \
             tc.tile_pool(name="rp", bufs=1) as rp, \
             tc.tile_pool(name="work", bufs=2) as workp:

            # ---------- constants / inputs ----------
            xf = []
            for c2 in range(2):
                t = constp.tile([128, N], F32R, name=f"xf{c2}", tag=f"xf{c2}")
                for off, w in CHUNKS_512:
                    nc.sync.dma_start(t[:, off:off + w],
                                      x_in.ap()[128 * c2:128 * (c2 + 1),
                                                off:off + w])
                xf.append(t)
            wqt, wkt, wvt = [], [], []
            for c2 in range(2):
                sl = slice(128 * c2, 128 * (c2 + 1))
                t = constp.tile([128, 128], F32R, name=f"wqt{c2}", tag=f"wqt{c2}")
                nc.sync.dma_start(t[:], wqt_d.ap()[sl, :])
                wqt.append(t)
                t = constp.tile([128, 128], F32R, name=f"wkt{c2}", tag=f"wkt{c2}")
                nc.sync.dma_start(t[:], wkt_d.ap()[sl, :])
                wkt.append(t)
                t = constp.tile([128, 512], F32R, name=f"wvt{c2}", tag=f"wvt{c2}")
                nc.sync.dma_start(t[:], wvt_d.ap()[sl, :])
                wvt.append(t)
            wpt = []
            for h in range(NHG):
                t = constp.tile([128, C], F32R, name=f"wpt{h}", tag=f"wpt{h}")
                nc.sync.dma_start(t[:], wpt_d.ap()[128 * h:128 * (h + 1), :])
                wpt.append(t)
            bq_t = constp.tile([128, 1], F32, name="bq_t", tag="bq_t")
            nc.sync.dma_start(bq_t[:], bq_d.ap())
            bk_t = constp.tile([128, 1], F32, name="bk_t", tag="bk_t")
            nc.sync.dma_start(bk_t[:], bk_d.ap())
            bv_t = []
            for h in range(NHG):
                t = constp.tile([128, 1], F32, name=f"bv{h}", tag=f"bv{h}")
                nc.sync.dma_start(t[:], bv_d.ap()[128 * h:128 * (h + 1), :])
                bv_t.append(t)
            # constants via DMA (memset can't produce float32r)
            consts_t = constp.tile([128, 128 + J], F32R, name="consts_t",
                                   tag="consts_t")
            nc.sync.dma_start(consts_t[:], consts_d.ap())
            ones_all = consts_t[:, 0:128]     # [128,128] of 1.0
            ones_bf = constp.tile([128, 1], BF16, name="ones_bf", tag="ones_bf")
            nc.sync.dma_start(ones_bf[:], ones_bf_d.ap())
            ones_t = consts_t[:, 0:1]         # [128,1] of 1.0
            zeros_row = consts_t[0:1, 128:128 + J]  # [1,J] of 0.0

            q_all = qkp.tile([128, N], F32R, name="q_all", tag="q_all")
            k_all = qkp.tile([128, N], F32R, name="k_all", tag="k_all")
            vt_all = vtp.tile([128, NMT * 512], BF16, name="vt_all", tag="vt_all")

            # ---------- phase 1: q/k/v projections ----------
            with tc.tile_pool(name="p1", bufs=4, space="PSUM") as p1:
                for off, w in CHUNKS_512:
                    ps = p1.tile([128, 512], F32, name="qproj", tag="p1")
                    for c2 in range(2):
                        nc.tensor.matmul(ps[:, :w], wqt[c2][:],
                                         xf[c2][:, off:off + w],
                                         start=(c2 == 0), stop=(c2 == 1))
                    nc.vector.tensor_scalar_add(q_all[:, off:off + w],
                                                ps[:, :w], bq_t[:])
                    ps = p1.tile([128, 512], F32, name="kproj", tag="p1")
                    for c2 in range(2):
                        nc.tensor.matmul(ps[:, :w], wkt[c2][:],
                                         xf[c2][:, off:off + w],
                                         start=(c2 == 0), stop=(c2 == 1))
                    nc.vector.tensor_scalar_add(k_all[:, off:off + w],
                                                ps[:, :w], bk_t[:])
                for nt in range(NMT):
                    ps = p1.tile([128, 512], F32, name="vproj", tag="p1")
                    for c2 in range(2):
                        nc.tensor.matmul(ps[:],
                                         xf[c2][:, 128 * nt:128 * (nt + 1)],
                                         wvt[c2][:],
                                         start=(c2 == 0), stop=(c2 == 1))
                    nc.vector.tensor_copy(vt_all[:, 512 * nt:512 * (nt + 1)],
                                          ps[:])

            # ---------- phase 2: attention + output projection ----------
            # n-chunks of width 512 (last 256); heads processed two at a
            # time (half-chunks). Per m-tile step, the two heads' S^T
            # matmuls land in the two banks of one [128,1024] psum tile
            # (alternating between two such tiles) and a single exp call
            # evicts both to a bf16 pexp tile. PV runs one step behind the
            # QKs so the PE never waits on ACT; consecutive pexp steps are
            # pair-summed on DVE (bf16 2x) into resident t-tiles, and the
            # denominator ones-matmuls + normalize + relu run in a finish
            # phase that uses two dedicated psum banks, overlapping the
            # next half-chunk's loop. The output projection needs all four
            # heads, so it runs after the second half-chunk.
            NP = NMT // 2
            JCHUNKS = [(2048, 256), (0, 512), (512, 512), (1024, 512),
                       (1536, 512)]
            with tc.tile_pool(name="stp", bufs=1, space="PSUM") as stp, \
                 tc.tile_pool(name="xxp", bufs=1, space="PSUM") as xxp, \
                 tc.tile_pool(name="finp", bufs=1, space="PSUM") as finp:
                for joff, JW in JCHUNKS:
                    r_ts = [None] * NHG
                    for ha, hb in ((0, 1), (2, 3)):
                        xx = {}
                        for h in (ha, hb):
                            xx[h] = xxp.tile([128, 512], F32, name=f"xx{h % 2}",
                                             tag=f"xx{h % 2}")[:, 0:JW]
                        pexp = [None] * NMT
                        tsum = [None] * NP

                        def emit_qk_exp(mt):
                            # the two heads' matmuls run concurrently
                            # (different row groups), so they must land in
                            # different psum banks: halves at 0 and 512.
                            st = stp.tile([128, 1024], F32, name="st",
                                          tag=f"st{mt % 2}")
                            moff = 128 * mt
                            for i, h in enumerate((ha, hb)):
                                nc.tensor.matmul(
                                    st[:, 512 * i:512 * i + JW],
                                    k_all[32 * h:32 * (h + 1), moff:moff + 128],
                                    q_all[32 * h:32 * (h + 1), joff:joff + JW],
                                    start=True, stop=True,
                                    tile_position=(32 * h, 0))
                            pe = pexpp.tile([128, 1024], BF16, name="pe",
                                            tag="pe", bufs=3)
                            if JW == 512:
                                nc.scalar.activation(pe[:, 0:1024],
                                                     st[:, 0:1024], AF.Exp)
                            else:
                                st_v = st.rearrange("p (a b) -> p a b",
                                                    b=512)[:, :, 0:JW]
                                pe_v = pe.rearrange("p (a b) -> p a b",
                                                    b=512)[:, :, 0:JW]
                                nc.scalar.activation(pe_v, st_v, AF.Exp)
                            pexp[mt] = pe

                        def emit_pv(mt):
                            pe = pexp[mt]
                            for i, h in enumerate((ha, hb)):
                                nc.tensor.matmul(
                                    xx[h],
                                    vt_all[:, 512 * mt + 128 * h:
                                           512 * mt + 128 * (h + 1)],
                                    pe[:, 512 * i:512 * i + JW],
                                    start=(mt == 0), stop=(mt == NMT - 1))

                        for mt in range(NMT):
                            emit_qk_exp(mt)
                            if mt > 0:
                                emit_pv(mt - 1)
                            if mt % 2 == 1:
                                k = mt // 2
                                t = workp.tile([128, 1024], BF16,
                                               name=f"t{k}", tag=f"t{k}",
                                               bufs=1)
                                nc.vector.tensor_tensor(
                                    t[:, 0:512 + JW],
                                    pexp[mt - 1][:, 0:512 + JW],
                                    pexp[mt][:, 0:512 + JW], OP.add)
                                tsum[k] = t
                        emit_pv(NMT - 1)

                        # finish: denominators, normalize, bias+relu
                        for i, h in enumerate((ha, hb)):
                            fslot = f"f{i}"
                            sums_h = finp.tile([1, 512], F32, name="sums_h",
                                               tag=fslot)
                            for k in range(NP):
                                nc.tensor.matmul(
                                    sums_h[:, 0:JW], ones_bf[:],
                                    tsum[k][:, 512 * i:512 * i + JW],
                                    start=(k == 0), stop=(k == NP - 1))
                            s_row = workp.tile([1, 512], F32R, name="s_row",
                                               tag="s_row")
                            nc.vector.tensor_copy(s_row[:, 0:JW],
                                                  sums_h[:, 0:JW])
                            s_bc = finp.tile([128, 512], F32, name="s_bc",
                                             tag=fslot)
                            nc.tensor.matmul(s_bc[:, 0:JW], ones_all[0:1, :],
                                             s_row[:, 0:JW],
                                             start=True, stop=True)
                            inv_s = workp.tile([128, 512], F32, name="inv_s",
                                               tag="inv_s")
                            nc.vector.reciprocal_approx_fast(inv_s[:, 0:JW],
                                                             s_bc[:, 0:JW])
                            t_h = workp.tile([128, 512], F32, name="t_h",
                                             tag="t_h")
                            nc.vector.tensor_tensor(t_h[:, 0:JW], xx[h],
                                                    inv_s[:, 0:JW], OP.mult)
                            r_h = rp.tile([128, 512], F32R, name=f"r{h}",
                                          tag=f"r{h}")
                            nc.scalar.activation(r_h[:, 0:JW], t_h[:, 0:JW],
                                                 AF.Relu, bias=bv_t[h][:])
                            r_ts[h] = r_h

                    # output projection over all four heads
                    for ct in range(2):
                        op_ps = finp.tile([128, 512], F32, name="op_ps",
                                          tag=f"f{ct}")
                        for h in range(NHG):
                            nc.tensor.matmul(
                                op_ps[:, 0:JW],
                                wpt[h][:, 128 * ct:128 * (ct + 1)],
                                r_ts[h][:, 0:JW],
                                start=(h == 0), stop=(h == NHG - 1))
                        o_sb = workp.tile([128, 512], F32, name="o_sb",
                                          tag="o_sb")
                        nc.vector.tensor_copy(o_sb[:, 0:JW], op_ps[:, 0:JW])
                        nc.sync.dma_start(
                            out_d.ap()[128 * ct:128 * (ct + 1), joff:joff + JW],
                            o_sb[:, 0:JW])
    nc.compile()
    return nc


def _prep_inputs(x, wq, gq, bq, wk, gk, bk, wv, gv, bv, wp, gp, bp):
    """Fold BN scales into weights; build the 8 per-core input maps."""
    rs = np.float32(1.0 / np.sqrt(np.float32(1.0) + np.float32(EPS)))
    sq = (gq * rs).astype(np.float32)
    sk = (gk * rs).astype(np.float32)
    sv = (gv * rs).astype(np.float32)
    sp = (gp * rs).astype(np.float32)
    wq_f = (wq * sq[:, None]).astype(np.float32)
    wk_f = (wk * sk[:, None]).astype(np.float32)
    wv_f = (wv * sv[:, None]).astype(np.float32)
    wp_f = (wp * sp[:, None]).astype(np.float32)

    xf = np.ascontiguousarray(x.reshape(B, C, N).astype(np.float32))
    consts = np.zeros((128, 128 + J), dtype=np.float32)
    consts[:, 0:128] = 1.0
    import ml_dtypes
    ones_bf = np.ones((128, 1), dtype=ml_dtypes.bfloat16)
    in_maps = []
    for core in range(8):
        b, g = core // 2, core % 2
        qs = slice(128 * g, 128 * (g + 1))       # q/k rows for this head group
        vs = slice(512 * g, 512 * (g + 1))       # v rows / p cols for this group
        in_maps.append({
            "x_in": xf[b],
            "wqt": np.ascontiguousarray(wq_f[qs, :].T),
            "wkt": np.ascontiguousarray(wk_f[qs, :].T),
            "wvt": np.ascontiguousarray(wv_f[vs, :].T),
            "wpt": np.ascontiguousarray(wp_f[:, vs].T),
            "bq": np.ascontiguousarray(bq[qs].astype(np.float32)[:, None]),
            "bk": np.ascontiguousarray(bk[qs].astype(np.float32)[:, None]),
            "bv": np.ascontiguousarray(bv[vs].astype(np.float32)[:, None]),
            "consts": consts,
            "ones_bf": ones_bf,
        })
    return in_maps


def kernel(**inputs):
    if "nc" not in _CACHE:
        _CACHE["nc"] = _build_program()
    nc = _CACHE["nc"]

    in_maps = _prep_inputs(**{k: np.asarray(v) for k, v in inputs.items()})
    res = run_bass_kernel_spmd(nc, in_maps, list(range(8)))
    _CACHE["last_results"] = res

    bp = np.asarray(inputs["bp"]).astype(np.float32)
    out = np.empty((B, C, H, W), dtype=np.float32)
    for b in range(B):
        acc = res.results[2 * b]["outp"] + res.results[2 * b + 1]["outp"]
        acc = acc + bp[:, None]
        out[b] = acc.reshape(C, H, W)
    return out


# revision 39
# speedup vs baseline: 1.0725x; 1.0725x over previous
"""Trainium2 Bass kernel for the BN-attention module (nn_Attention).

Full inputs -> full output. Sharding: 8 cores = (batch b in 0..3) x
(head-group g in 0..1, 4 heads each). Each core computes its batch's
4-head attention and a partial output projection; the host sums the two
head-group partials per batch and adds the projection BN bias.

Numerics: BN scales are folded into the weights on the host. QK^T and
the projections run as float32r (fp32 storage, FP22 multiply, fp32
accumulate; full PE rate at free dim >= 256). Softmax skips the
max-subtraction (logits are O(25) here, safe in fp32); exp(S^T) is
stored bf16, and both attn@V and the softmax denominators consume the
same bf16 values so their rounding largely cancels in the ratio.

Layout: attention is computed transposed, S^T = K^T Q with keys (m) on
partitions, so attn@V needs no transposes at all: V is produced
directly as vT[n,d] by the projection, and the denominators are
column sums done with ones-matmuls on the TensorE (plus one level of
bf16 pair-summing on the VectorE, which runs at 2x for bf16).
Normalization, BN-v bias, and ReLU fold into the PSUM eviction of xx.
"""

import numpy as np

import concourse.bacc as bacc
import concourse.mybir as mybir
import concourse.tile as tile
from concourse.bass_utils import run_bass_kernel_spmd

# Problem dims (hardcoded per the spec)
B, C, H, W = 4, 256, 48, 48
N = H * W            # 2304
KD, NH, AR = 32, 8, 4
D = AR * KD          # 128 value dims per head
NHKD = NH * KD       # 256
DH = NH * D          # 1024
EPS = 1e-5

NHG = 4              # heads per core
J = 256              # n-chunk width
NJ = N // J          # 9
MT = 128             # m-tile (key tile)
NMT = N // MT        # 18

F32 = mybir.dt.float32
F32R = mybir.dt.float32r
BF16 = mybir.dt.bfloat16
AF = mybir.ActivationFunctionType
OP = mybir.AluOpType

# chunks of 2304 by <=512 for the projection matmuls
CHUNKS_512 = [(off, min(512, N - off)) for off in range(0, N, 512)]

_CACHE = {}


def _build_program():
    nc = bacc.Bacc("TRN2", target_bir_lowering=False, debug=False)

    x_in = nc.dram_tensor("x_in", [C, N], F32R, kind="ExternalInput")
    wqt_d = nc.dram_tensor("wqt", [C, 128], F32R, kind="ExternalInput")
    wkt_d = nc.dram_tensor("wkt", [C, 128], F32R, kind="ExternalInput")
    wvt_d = nc.dram_tensor("wvt", [C, 512], F32R, kind="ExternalInput")
    wpt_d = nc.dram_tensor("wpt", [512, C], F32R, kind="ExternalInput")
    bq_d = nc.dram_tensor("bq", [128, 1], F32, kind="ExternalInput")
    bk_d = nc.dram_tensor("bk", [128, 1], F32, kind="ExternalInput")
    bv_d = nc.dram_tensor("bv", [512, 1], F32, kind="ExternalInput")
    consts_d = nc.dram_tensor("consts", [128, 128 + J], F32R, kind="ExternalInput")
    ones_bf_d = nc.dram_tensor("ones_bf", [128, 1], BF16, kind="ExternalInput")
    out_d = nc.dram_tensor("outp", [C, N], F32, kind="ExternalOutput")

    with tile.TileContext(nc) as tc:
        with nc.allow_low_precision(reason="float32r rounding is intentional"), \
             tc.tile_pool(name="const", bufs=1) as constp, \
             tc.tile_pool(name="qk", bufs=1) as qkp, \
             tc.tile_pool(name="vt", bufs=1) as vtp, \
             tc.tile_pool(name="pexp", bufs=1) as pexpp, \
             tc.tile_pool(name="rp", bufs=1) as rp, \
             tc.tile_pool(name="work", bufs=2) as workp:

            # ---------- constants / inputs ----------
            xf = []
            for c2 in range(2):
                t = constp.tile([128, N], F32R, name=f"xf{c2}", tag=f"xf{c2}")
                for off, w in CHUNKS_512:
                    nc.sync.dma_start(t[:, off:off + w],
                                      x_in.ap()[128 * c2:128 * (c2 + 1),
                                                off:off + w])
                xf.append(t)
            wqt, wkt, wvt = [], [], []
            for c2 in range(2):
                sl = slice(128 * c2, 128 * (c2 + 1))
                t = constp.tile([128, 128], F32R, name=f"wqt{c2}", tag=f"wqt{c2}")
                nc.sync.dma_start(t[:], wqt_d.ap()[sl, :])
                wqt.append(t)
                t = constp.tile([128, 128], F32R, name=f"wkt{c2}", tag=f"wkt{c2}")
                nc.sync.dma_start(t[:], wkt_d.ap()[sl, :])
                wkt.append(t)
                t = constp.tile([128, 512], F32R, name=f"wvt{c2}", tag=f"wvt{c2}")
                nc.sync.dma_start(t[:], wvt_d.ap()[sl, :])
                wvt.append(t)
            wpt = []
            for h in range(NHG):
                t = constp.tile([128, C], F32R, name=f"wpt{h}", tag=f"wpt{h}")
                nc.sync.dma_start(t[:], wpt_d.ap()[128 * h:128 * (h + 1), :])
                wpt.append(t)
            bq_t = constp.tile([128, 1], F32, name="bq_t", tag="bq_t")
            nc.sync.dma_start(bq_t[:], bq_d.ap())
            bk_t = constp.tile([128, 1], F32, name="bk_t", tag="bk_t")
            nc.sync.dma_start(bk_t[:], bk_d.ap())
            bv_t = []
            for h in range(NHG):
                t = constp.tile([128, 1], F32, name=f"bv{h}", tag=f"bv{h}")
                nc.sync.dma_start(t[:], bv_d.ap()[128 * h:128 * (h + 1), :])
                bv_t.append(t)
            # constants via DMA (memset can't produce float32r)
            consts_t = constp.tile([128, 128 + J], F32R, name="consts_t",
                                   tag="consts_t")
            nc.sync.dma_start(consts_t[:], consts_d.ap())
            ones_all = consts_t[:, 0:128]     # [128,128] of 1.0
            ones_bf = constp.tile([128, 1], BF16, name="ones_bf", tag="ones_bf")
            nc.sync.dma_start(ones_bf[:], ones_bf_d.ap())
            ones_t = consts_t[:, 0:1]         # [128,1] of 1.0
            zeros_row = consts_t[0:1, 128:128 + J]  # [1,J] of 0.0

            q_all = qkp.tile([128, N], F32R, name="q_all", tag="q_all")
            k_all = qkp.tile([128, N], F32R, name="k_all", tag="k_all")
            vt_all = vtp.tile([128, NMT * 512], BF16, name="vt_all", tag="vt_all")

            # ---------- phase 1: q/k/v projections ----------
            with tc.tile_pool(name="p1", bufs=4, space="PSUM") as p1:
                for off, w in CHUNKS_512:
                    ps = p1.tile([128, 512], F32, name="qproj", tag="p1")
                    for c2 in range(2):
                        nc.tensor.matmul(ps[:, :w], wqt[c2][:],
                                         xf[c2][:, off:off + w],
                                         start=(c2 == 0), stop=(c2 == 1))
                    nc.vector.tensor_scalar_add(q_all[:, off:off + w],
                                                ps[:, :w], bq_t[:])
                    ps = p1.tile([128, 512], F32, name="kproj", tag="p1")
                    for c2 in range(2):
                        nc.tensor.matmul(ps[:, :w], wkt[c2][:],
                                         xf[c2][:, off:off + w],
                                         start=(c2 == 0), stop=(c2 == 1))
                    nc.vector.tensor_scalar_add(k_all[:, off:off + w],
                                                ps[:, :w], bk_t[:])
                for nt in range(NMT):
                    ps = p1.tile([128, 512], F32, name="vproj", tag="p1")
                    for c2 in range(2):
                        nc.tensor.matmul(ps[:],
                                         xf[c2][:, 128 * nt:128 * (nt + 1)],
                                         wvt[c2][:],
                                         start=(c2 == 0), stop=(c2 == 1))
                    nc.vector.tensor_copy(vt_all[:, 512 * nt:512 * (nt + 1)],
                                          ps[:])

            # ---------- phase 2: attention + output projection ----------
            # n-chunks of width 512 (last 256); heads processed two at a
            # time (half-chunks). Per m-tile step, the two heads' S^T
            # matmuls land in the two banks of one [128,1024] psum tile
            # (alternating between two such tiles) and a single exp call
            # evicts both to a bf16 pexp tile. PV runs one step behind the
            # QKs so the PE never waits on ACT; consecutive pexp steps are
            # pair-summed on DVE (bf16 2x) into resident t-tiles, and the
            # denominator ones-matmuls + normalize + relu run in a finish
            # phase that uses two dedicated psum banks, overlapping the
            # next half-chunk's loop. The output projection needs all four
            # heads, so it runs after the second half-chunk.
            NP = NMT // 2
            JCHUNKS = [(0, 512), (512, 512), (1024, 512), (1536, 512),
                       (2048, 256)]
            with tc.tile_pool(name="stp", bufs=1, space="PSUM") as stp, \
                 tc.tile_pool(name="xxp", bufs=1, space="PSUM") as xxp, \
                 tc.tile_pool(name="finp", bufs=1, space="PSUM") as finp:
                for joff, JW in JCHUNKS:
                    r_ts = [None] * NHG
                    for ha, hb in ((0, 1), (2, 3)):
                        xx = {}
                        for h in (ha, hb):
                            xx[h] = xxp.tile([128, 512], F32, name=f"xx{h % 2}",
                                             tag=f"xx{h % 2}")[:, 0:JW]
                        pexp = [None] * NMT
                        tsum = [None] * NP

                        def emit_qk_exp(mt):
                            # the two heads' matmuls run concurrently
                            # (different row groups), so they must land in
                            # different psum banks: halves at 0 and 512.
                            st = stp.tile([128, 1024], F32, name="st",
                                          tag=f"st{mt % 2}")
                            moff = 128 * mt
                            for i, h in enumerate((ha, hb)):
                                nc.tensor.matmul(
                                    st[:, 512 * i:512 * i + JW],
                                    k_all[32 * h:32 * (h + 1), moff:moff + 128],
                                    q_all[32 * h:32 * (h + 1), joff:joff + JW],
                                    start=True, stop=True,
                                    tile_position=(32 * h, 0))
                            pe = pexpp.tile([128, 1024], BF16, name="pe",
                                            tag="pe", bufs=3)
                            if JW == 512:
                                nc.scalar.activation(pe[:, 0:1024],
                                                     st[:, 0:1024], AF.Exp)
                            else:
                                st_v = st.rearrange("p (a b) -> p a b",
                                                    b=512)[:, :, 0:JW]
                                pe_v = pe.rearrange("p (a b) -> p a b",
                                                    b=512)[:, :, 0:JW]
                                nc.scalar.activation(pe_v, st_v, AF.Exp)
                            pexp[mt] = pe

                        def emit_pv(mt):
                            pe = pexp[mt]
                            for i, h in enumerate((ha, hb)):
                                nc.tensor.matmul(
                                    xx[h],
                                    vt_all[:, 512 * mt + 128 * h:
                                           512 * mt + 128 * (h + 1)],
                                    pe[:, 512 * i:512 * i + JW],
                                    start=(mt == 0), stop=(mt == NMT - 1))

                        for mt in range(NMT):
                            emit_qk_exp(mt)
                            if mt > 0:
                                emit_pv(mt - 1)
                            if mt % 2 == 1:
                                k = mt // 2
                                t = workp.tile([128, 1024], BF16,
                                               name=f"t{k}", tag=f"t{k}",
                                               bufs=1)
                                nc.vector.tensor_tensor(
                                    t[:, 0:512 + JW],
                                    pexp[mt - 1][:, 0:512 + JW],
                                    pexp[mt][:, 0:512 + JW], OP.add)
                                tsum[k] = t
                        emit_pv(NMT - 1)

                        # finish: denominators, normalize, bias+relu
                        for i, h in enumerate((ha, hb)):
                            fslot = f"f{i}"
                            sums_h = finp.tile([1, 512], F32, name="sums_h",
                                               tag=fslot)
                            for k in range(NP):
                                nc.tensor.matmul(
                                    sums_h[:, 0:JW], ones_bf[:],
                                    tsum[k][:, 512 * i:512 * i + JW],
                                    start=(k == 0), stop=(k == NP - 1))
                            s_row = workp.tile([1, 512], F32R, name="s_row",
                                               tag="s_row")
                            nc.vector.tensor_copy(s_row[:, 0:JW],
                                                  sums_h[:, 0:JW])
                            s_bc = finp.tile([128, 512], F32, name="s_bc",
                                             tag=fslot)
                            nc.tensor.matmul(s_bc[:, 0:JW], ones_all[0:1, :],
                                             s_row[:, 0:JW],
                                             start=True, stop=True)
                            inv_s = workp.tile([128, 512], F32, name="inv_s",
                                               tag="inv_s")
                            nc.vector.reciprocal_approx_fast(inv_s[:, 0:JW],
                                                             s_bc[:, 0:JW])
                            t_h = workp.tile([128, 512], F32, name="t_h",
                                             tag="t_h")
                            nc.vector.tensor_tensor(t_h[:, 0:JW], xx[h],
                                                    inv_s[:, 0:JW], OP.mult)
                            r_h = rp.tile([128, 512], F32R, name=f"r{h}",
                                          tag=f"r{h}")
                            nc.scalar.activation(r_h[:, 0:JW], t_h[:, 0:JW],
                                                 AF.Relu, bias=bv_t[h][:])
                            r_ts[h] = r_h

                    # output projection over all four heads
                    for ct in range(2):
                        op_ps = finp.tile([128, 512], F32, name="op_ps",
                                          tag=f"f{ct}")
                        for h in range(NHG):
                            nc.tensor.matmul(
                                op_ps[:, 0:JW],
                                wpt[h][:, 128 * ct:128 * (ct + 1)],
                                r_ts[h][:, 0:JW],
                                start=(h == 0), stop=(h == NHG - 1))
                        o_sb = workp.tile([128, 512], F32, name="o_sb",
                                          tag="o_sb")
                        nc.vector.tensor_copy(o_sb[:, 0:JW], op_ps[:, 0:JW])
                        nc.sync.dma_start(
                            out_d.ap()[128 * ct:128 * (ct + 1), joff:joff + JW],
                            o_sb[:, 0:JW])
    nc.compile()
    return nc


def _prep_inputs(x, wq, gq, bq, wk, gk, bk, wv, gv, bv, wp, gp, bp):
    """Fold BN scales into weights; build the 8 per-core input maps."""
    rs = np.float32(1.0 / np.sqrt(np.float32(1.0) + np.float32(EPS)))
    sq = (gq * rs).astype(np.float32)
    sk = (gk * rs).astype(np.float32)
    sv = (gv * rs).astype(np.float32)
    sp = (gp * rs).astype(np.float32)
    wq_f = (wq * sq[:, None]).astype(np.float32)
    wk_f = (wk * sk[:, None]).astype(np.float32)
    wv_f = (wv * sv[:, None]).astype(np.float32)
    wp_f = (wp * sp[:, None]).astype(np.float32)

    xf = np.ascontiguousarray(x.reshape(B, C, N).astype(np.float32))
    consts = np.zeros((128, 128 + J), dtype=np.float32)
    consts[:, 0:128] = 1.0
    import ml_dtypes
    ones_bf = np.ones((128, 1), dtype=ml_dtypes.bfloat16)
    in_maps = []
    for core in range(8):
        b, g = core // 2, core % 2
        qs = slice(128 * g, 128 * (g + 1))       # q/k rows for this head group
        vs = slice(512 * g, 512 * (g + 1))       # v rows / p cols for this group
        in_maps.append({
            "x_in": xf[b],
            "wqt": np.ascontiguousarray(wq_f[qs, :].T),
            "wkt": np.ascontiguousarray(wk_f[qs, :].T),
            "wvt": np.ascontiguousarray(wv_f[vs, :].T),
            "wpt": np.ascontiguousarray(wp_f[:, vs].T),
            "bq": np.ascontiguousarray(bq[qs].astype(np.float32)[:, None]),
            "bk": np.ascontiguousarray(bk[qs].astype(np.float32)[:, None]),
            "bv": np.ascontiguousarray(bv[vs].astype(np.float32)[:, None]),
            "consts": consts,
            "ones_bf": ones_bf,
        })
    return in_maps


def kernel(**inputs):
    if "nc" not in _CACHE:
        _CACHE["nc"] = _build_program()
    nc = _CACHE["nc"]

    in_maps = _prep_inputs(**{k: np.asarray(v) for k, v in inputs.items()})
    res = run_bass_kernel_spmd(nc, in_maps, list(range(8)))
    _CACHE["last_results"] = res

    bp = np.asarray(inputs["bp"]).astype(np.float32)
    out = np.empty((B, C, H, W), dtype=np.float32)
    for b in range(B):
        acc = res.results[2 * b]["outp"] + res.results[2 * b + 1]["outp"]
        acc = acc + bp[:, None]
        out[b] = acc.reshape(C, H, W)
    return out
